# revision 7
# baseline (speedup 1.0000x reference)
"""Pre-LN causal attention with bias, sharded over 8 TRN2 NeuronCores.

Sharding: (batch, head-group) — core c handles batch c//4 and heads
[(c%4)*4 : (c%4)*4+4].  Each core computes LN -> q/k/v projections for its
head group -> biased causal attention -> partial output projection
(row-sharded wo).  Host sums the 4 partials per batch (the unshard for a
row-sharded to_out).

Device pipeline is in "transposed" layout so no on-chip transpose of the
big score matrix is ever needed:
  xn[tok,dim] -(PE transpose)-> xnT[dim,tok]
  qT/kT = w.T @ xnT          [256, 2048]
  v     = xn @ wv            [2048, 260]  (65th column per head = ones)
  ST    = kT.T @ qT          [j, i] blocks, + biasT (host pre-transposed)
  PT    = exp(ST)            (no max subtraction; logits bounded ~N(0,2))
  OT    = V_aug.T @ PT       row 64 = softmax denominator r
  Y    += (OT/r).T @ wo      accumulated over 4 heads
Causal: blocks with i<j skipped entirely (compute + bias DMA), diagonal
128x128 sub-block masked with an additive -1e30 constant tile.
"""

import sys

sys.path.insert(0, "/opt/trn_rl_repo")

import numpy as np
import ml_dtypes

B = 2
N = 2048
DIM = 1024
HEADS = 16
D = 64
INNER = HEADS * D
HL = 4          # heads per core
GCOLS = HL * D  # 256 projection cols per core
NCORES = 8
SCALE = D ** -0.5
LN_EPS = 1e-5
NT = N // 128   # 16 token tiles
KT = DIM // 128  # 8 dim tiles
NIB = N // 512  # 4 i-blocks
NEG = -1.0e30

_CACHE = {}


def _build_program():
    import concourse.bacc as bacc
    import concourse.mybir as mybir
    import concourse.tile as tile

    FP = mybir.dt.float32
    BF = mybir.dt.bfloat16
    AX = mybir.AxisListType.X
    AF = mybir.ActivationFunctionType

    nc = bacc.Bacc("TRN2", target_bir_lowering=False, debug=False,
                   num_devices=NCORES)

    x_d = nc.dram_tensor("x", (N, DIM), FP, kind="ExternalInput")
    wq_d = nc.dram_tensor("wq", (DIM, GCOLS), FP, kind="ExternalInput")
    wk_d = nc.dram_tensor("wk", (DIM, GCOLS), FP, kind="ExternalInput")
    wv_d = nc.dram_tensor("wv", (DIM, GCOLS), FP, kind="ExternalInput")
    wo_d = nc.dram_tensor("wo", (GCOLS, DIM), FP, kind="ExternalInput")
    bT_d = nc.dram_tensor("biasT", (HL, N, N), BF, kind="ExternalInput")
    cm_d = nc.dram_tensor("cmask", (128, 128), FP, kind="ExternalInput")
    id_d = nc.dram_tensor("ident", (128, 128), FP, kind="ExternalInput")
    on_d = nc.dram_tensor("ones64", (1, 64), FP, kind="ExternalInput")
    out_d = nc.dram_tensor("out", (N, DIM), FP, kind="ExternalOutput")

    with tile.TileContext(nc) as tc:
        with (
            tc.tile_pool(name="const", bufs=1) as cp,
            tc.tile_pool(name="xload", bufs=3) as xp,
            tc.tile_pool(name="ln", bufs=3) as lnp,
            tc.tile_pool(name="stats", bufs=4) as stp,
            tc.tile_pool(name="persist", bufs=1) as pp,
            tc.tile_pool(name="bias", bufs=4) as bp,
            tc.tile_pool(name="pt", bufs=6) as ptp,
            tc.tile_pool(name="yout", bufs=3) as yp,
            tc.tile_pool(name="ps", bufs=2, space="PSUM") as psp,
        ):
            # ---- constants in SBUF
            ident = cp.tile_from(id_d[:, :], dtype=BF, name="identb")
            cmask = cp.tile_from(cm_d[:, :], name="cmaskb")
            ones64 = cp.tile_from(on_d[:, :], name="ones64b")
            epsb = cp.tile([128, 1], FP, name="epsb")
            nc.vector.memset(epsb, LN_EPS)
            zerob = cp.tile([128, 1], FP, name="zerob")
            nc.vector.memset(zerob, 0.0)
            wq_sb = [cp.tile_from(wq_d[k * 128:(k + 1) * 128, :], dtype=BF,
                                  name=f"wq{k}") for k in range(KT)]
            wk_sb = [cp.tile_from(wk_d[k * 128:(k + 1) * 128, :], dtype=BF,
                                  name=f"wk{k}") for k in range(KT)]
            wv_sb = [cp.tile_from(wv_d[k * 128:(k + 1) * 128, :], dtype=BF,
                                  name=f"wv{k}") for k in range(KT)]
            wo_sb = [cp.tile_from(wo_d[h * 64:(h + 1) * 64, :], dtype=BF,
                                  name=f"wo{h}") for h in range(HL)]

            # ---- persistent activations
            xnT = [pp.tile([128, N], BF, name=f"xnT{k}") for k in range(KT)]
            qT = [pp.tile([128, N], BF, name=f"qT{m}") for m in range(2)]
            kTt = [pp.tile([128, N], BF, name=f"kT{m}") for m in range(2)]
            v_sb = [pp.tile([128, HL * 65], BF, name=f"v{t}")
                    for t in range(NT)]
            onrm = [pp.tile([64, N], BF, name=f"on{h}") for h in range(HL)]

            # ---- phase 1: LayerNorm + transpose
            for t in range(NT):
                x_t = xp.tile([128, DIM], FP, tag="x")
                nc.sync.dma_start(x_t, x_d[t * 128:(t + 1) * 128, :])
                ssum = stp.tile([128, 1], FP, tag="ssum")
                nc.vector.reduce_sum(out=ssum, in_=x_t, axis=AX)
                sq = lnp.tile([128, DIM], FP, tag="sq")
                ssq = stp.tile([128, 1], FP, tag="ssq")
                nc.scalar.activation(out=sq, in_=x_t, func=AF.Square,
                                     bias=zerob[:, :], accum_out=ssq)
                mean = stp.tile([128, 1], FP, tag="mean")
                nc.vector.tensor_scalar_mul(mean, ssum, 1.0 / DIM)
                ex2 = stp.tile([128, 1], FP, tag="ex2")
                nc.vector.tensor_scalar_mul(ex2, ssq, 1.0 / DIM)
                msq = stp.tile([128, 1], FP, tag="msq")
                nc.vector.tensor_mul(msq, mean, mean)
                var = stp.tile([128, 1], FP, tag="var")
                nc.vector.tensor_sub(var, ex2, msq)
                std = stp.tile([128, 1], FP, tag="std")
                nc.scalar.activation(out=std, in_=var, func=AF.Sqrt,
                                     bias=epsb[:, :])
                rsig = stp.tile([128, 1], FP, tag="rsig")
                nc.vector.reciprocal(rsig, std)
                xn = lnp.tile([128, DIM], BF, tag="xn")
                nc.vector.tensor_scalar(xn, x_t, mean, rsig,
                                        op0=mybir.AluOpType.subtract,
                                        op1=mybir.AluOpType.mult)
                for k in range(KT):
                    tp = psp.tile([128, 128], BF, tag="tr", bufs=2)
                    nc.tensor.transpose(tp, xn[:, k * 128:(k + 1) * 128],
                                        ident)
                    nc.scalar.copy(out=xnT[k][:, t * 128:(t + 1) * 128],
                                   in_=tp)

            # ---- phase 2: qT / kT projections ([256, N] each, 2 m-tiles)
            for dst, w_sb in ((qT, wq_sb), (kTt, wk_sb)):
                for m in range(2):
                    for nb in range(NIB):
                        ps = psp.tile([128, 512], FP, tag="mm", bufs=2)
                        for k in range(KT):
                            nc.tensor.matmul(
                                ps,
                                lhsT=w_sb[k][:, m * 128:(m + 1) * 128],
                                rhs=xnT[k][:, nb * 512:(nb + 1) * 512],
                                start=(k == 0), stop=(k == KT - 1))
                        nc.scalar.copy(
                            out=dst[m][:, nb * 512:(nb + 1) * 512], in_=ps)

            # ---- phase 3: v in natural layout, ones-augmented per head
            for t in range(NT):
                ps = psp.tile([128, 512], FP, tag="sc", bufs=2)
                for k in range(KT):
                    nc.tensor.matmul(
                        ps[:, 0:GCOLS],
                        lhsT=xnT[k][:, t * 128:(t + 1) * 128],
                        rhs=wv_sb[k],
                        start=(k == 0), stop=(k == KT - 1))
                for h in range(HL):
                    nc.scalar.copy(out=v_sb[t][:, h * 65:h * 65 + 64],
                                   in_=ps[:, h * 64:(h + 1) * 64])
                    nc.vector.memset(v_sb[t][:, h * 65 + 64:h * 65 + 65], 1.0)

            # ---- phase 4: attention, transposed-score layout
            for ib in range(NIB):
                njt = 4 * ib + 4
                for h in range(HL):
                    mq = h // 2
                    r0 = (h % 2) * 64
                    ops = psp.tile([65, 512], FP, tag="o", bufs=2)
                    for jt in range(njt):
                        scps = psp.tile([128, 512], FP, tag="sc", bufs=2)
                        nc.tensor.matmul(
                            scps,
                            lhsT=kTt[mq][r0:r0 + 64,
                                         jt * 128:(jt + 1) * 128],
                            rhs=qT[mq][r0:r0 + 64,
                                       ib * 512:(ib + 1) * 512],
                            start=True, stop=True)
                        pt = ptp.tile([128, 512], BF, tag="pt")
                        p = jt - 4 * ib
                        i0 = max(0, p * 128)
                        w = 512 - i0
                        bt = bp.tile([128, 512], BF, tag="bias")
                        nc.sync.dma_start(
                            bt[:, 0:w],
                            bT_d[h, jt * 128:(jt + 1) * 128,
                                 ib * 512 + i0:(ib + 1) * 512])
                        sb = bp.tile([128, 512], FP, tag="sb")
                        nc.vector.tensor_add(sb[:, 0:w], scps[:, i0:512],
                                             bt[:, 0:w])
                        if p >= 0:
                            # diagonal j-tile: mask 128-wide diag sub-block,
                            # zero the fully-masked left region
                            nc.vector.tensor_add(sb[:, 0:128], sb[:, 0:128],
                                                 cmask)
                            if i0 > 0:
                                nc.vector.memset(pt[:, 0:i0], 0.0)
                        nc.scalar.activation(out=pt[:, i0:512],
                                             in_=sb[:, 0:w], func=AF.Exp,
                                             bias=zerob[:, :])
                        nc.tensor.matmul(
                            ops,
                            lhsT=v_sb[jt][:, h * 65:h * 65 + 65],
                            rhs=pt,
                            start=(jt == 0), stop=(jt == njt - 1))
                    # normalize: r = row 64 of ops
                    rc = stp.tile([1, 512], FP, tag="rc")
                    nc.vector.reciprocal(rc, ops[64:65, :])
                    reps = psp.tile([64, 512], FP, tag="sc", bufs=2)
                    nc.tensor.matmul(reps, lhsT=ones64, rhs=rc,
                                     start=True, stop=True)
                    rep_sb = stp.tile([64, 512], FP, tag="repsb")
                    nc.scalar.copy(rep_sb, reps)
                    nc.vector.tensor_mul(
                        onrm[h][:, ib * 512:(ib + 1) * 512],
                        ops[0:64, :], rep_sb)

            # ---- phase 5: output projection (partial over this head group)
            for t in range(NT):
                for nb in range(2):
                    yps = psp.tile([128, 512], FP, tag="mm", bufs=2)
                    for h in range(HL):
                        nc.tensor.matmul(
                            yps,
                            lhsT=onrm[h][:, t * 128:(t + 1) * 128],
                            rhs=wo_sb[h][:, nb * 512:(nb + 1) * 512],
                            start=(h == 0), stop=(h == HL - 1))
                    y = yp.tile([128, 512], FP, tag="y")
                    nc.scalar.copy(y, yps)
                    nc.sync.dma_start(
                        out_d[t * 128:(t + 1) * 128,
                              nb * 512:(nb + 1) * 512], y)

    nc.compile()
    return nc


def _get_program():
    if "nc" not in _CACHE:
        _CACHE["nc"] = _build_program()
    return _CACHE["nc"]


def _make_in_maps(x, attn_bias, gamma, beta, wq, wkv, wo):
    x = np.asarray(x, np.float32)
    attn_bias = np.asarray(attn_bias, np.float32)
    gamma = np.asarray(gamma, np.float32)
    wq = np.asarray(wq, np.float32) * gamma[:, None]
    wkv = np.asarray(wkv, np.float32) * gamma[:, None]
    wo = np.asarray(wo, np.float32)

    jj, ii = np.mgrid[0:128, 0:128]
    cmask = np.where(jj > ii, NEG, 0.0).astype(np.float32)
    ident = np.eye(128, dtype=np.float32)
    ones64 = np.ones((1, 64), np.float32)

    in_maps = []
    for c in range(NCORES):
        b = c // 4
        g = c % 4
        cols = slice(g * GCOLS, (g + 1) * GCOLS)
        biasT = np.ascontiguousarray(
            attn_bias[g * HL:(g + 1) * HL].transpose(0, 2, 1)
        ).astype(ml_dtypes.bfloat16)
        in_maps.append({
            "x": np.ascontiguousarray(x[b]),
            "wq": np.ascontiguousarray(wq[:, cols]) * SCALE,
            "wk": np.ascontiguousarray(wkv[:, cols]),
            "wv": np.ascontiguousarray(wkv[:, INNER:][:, cols]),
            "wo": np.ascontiguousarray(wo[cols, :]),
            "biasT": biasT,
            "cmask": cmask,
            "ident": ident,
            "ones64": ones64,
        })
    return in_maps


def run(inputs, trace=False):
    from concourse import bass_utils
    nc = _get_program()
    in_maps = _make_in_maps(**inputs)
    res = bass_utils.run_bass_kernel_spmd(
        nc, in_maps, core_ids=list(range(NCORES)), trace=trace)
    outs = [np.asarray(res.results[c]["out"], np.float32)
            for c in range(NCORES)]
    full = np.stack([outs[0] + outs[1] + outs[2] + outs[3],
                     outs[4] + outs[5] + outs[6] + outs[7]])
    return full, res


def kernel(**inputs):
    full, _ = run(inputs, trace=False)
    return full
